# revision 1
# baseline (speedup 1.0000x reference)
"""Trainium2 Bass kernel for the S-LSTM (sentence-state LSTM) classifier.

Data-parallel over batch: 8 cores x 4 examples. Everything on-chip runs in a
"transposed" layout: feature channels on SBUF partitions, (example, position)
flattened on the free dim (4*128 = 512 columns). The per-step gate GEMM
computes gates.T = Wg.T @ ctx.T with Wg slices stationary and h.T moving at
N=512 (fp32 data in float32r mode -> full PE rate). Position shifts
(h_{i-1}, h_{i+1}, c shifts) are free-dim offsets into state tiles that carry
one zero guard column on each side of every example's 128 columns.

The global-node ("g") part of ctx is rank-1 along positions: gg = g @ Wg_g is
computed once per step as a tiny M=4 GEMM, then folded into the big GEMM as an
extra K chunk against a constant 0/1 selector matrix (zero-padded to K=128).

Weights are streamed from HBM each step in host-pre-tiled contiguous pieces
(one DMA per stationary block column), overlapped with PE work.
"""

import ml_dtypes
import numpy as np

import concourse.bass as bass
import concourse.mybir as mybir
from concourse import bacc
import concourse.tile as tile
from concourse.bass_utils import run_bass_kernel_spmd

F32 = mybir.dt.float32
F32R = mybir.dt.float32r
F16 = mybir.dt.float16
BF16 = mybir.dt.bfloat16
I32 = mybir.dt.int32
AL = mybir.AluOpType
AF = mybir.ActivationFunctionType
AX = mybir.AxisListType

B, L, V, E, H, DOUT = 32, 128, 30000, 300, 512, 5
NUM_STEPS = 5
NCORES = 8
BL = B // NCORES          # 4 examples per core
N = BL * L                # 512 free columns
EP = 384                  # E padded to 3*128
HC = H // 128             # 4 H chunks
GC = 7 * H // 128         # 28 gate output chunks
KHH = 3 * HC              # 12 K chunks for hl/h/hr
EC = EP // 128            # 3 E chunks
GG_W = 7 * H + H          # 4096: [Wg_g | Wfi_g] columns
GGC = GG_W // 512         # 8
DP = 8                    # DOUT padded to even size for fp32r matmul


def build_nc():
    nc = bacc.Bacc(trn_type="TRN2", target_bir_lowering=False)

    d = {}

    def din(name, shape, dt=F32):
        d[name] = nc.dram_tensor(name, list(shape), dt, kind="ExternalInput")
        return d[name]

    # weights are host-pre-tiled so every DMA reads contiguous HBM
    embed_d = din("embed", (V, E))
    wg_hhh = din("wg_hhh", (GC, 128, KHH, 128), BF16)
    wg_x = din("wg_x", (GC, 128, EC, 128), F32R)
    wg_gcat = din("wg_gcat", (HC, GGC, 128, 512), BF16)
    wfi_h = din("wfi_h", (128, HC, H), BF16)
    wgf_d = din("wgf", (HC, 128, 2 * HC, 128), BF16)
    wgo_d = din("wgo", (HC, 128, 2 * HC, 128), BF16)
    w0_d = din("w0", (HC, 128, EC, 128), F32R)
    w1_d = din("w1", (2 * HC, 128, HC, 128), BF16)
    w2_d = din("w2", (128, 2 * HC, DP), BF16)
    bg_t = din("bg_t", (128, GC))
    b0_t = din("b0_t", (128, HC))
    bfi_t = din("bfi_t", (128, HC))
    bgf_t = din("bgf_t", (128, HC))
    bgo_t = din("bgo_t", (128, HC))
    b1_t = din("b1_t", (128, 2 * HC))
    b2_r = din("b2_r", (BL, DP))
    sel_d = din("sel", (128, N), F32R)
    ident_d = din("ident", (128, 128))
    tok_d = din("tok_idx", (128, BL), I32)       # column e = tokens of example e
    mask_d = din("mask_rep", (128, N))
    invlen_d = din("invlen_rep", (128, BL))

    out_d = nc.dram_tensor("out", [BL, DOUT], F32, kind="ExternalOutput")

    with tile.TileContext(nc) as tc:
        with (
            tc.tile_pool(name="psumA", bufs=4, space="PSUM") as psumA,
            tc.tile_pool(name="psumB", bufs=2, space="PSUM") as psumB,
            tc.tile_pool(name="psumT", bufs=2, space="PSUM") as psumT,
            tc.tile_pool(name="gates", bufs=10) as p_gate,
            tc.tile_pool(name="tmp", bufs=12) as p_tmp,
            tc.tile_pool(name="wg", bufs=6) as p_wg,
            tc.tile_pool(name="wcat", bufs=6) as p_wcat,
            tc.tile_pool(name="wgfgo", bufs=4) as p_wgfgo,
            tc.tile_pool(name="wx", bufs=3) as p_wx,
            tc.tile_pool(name="w1p", bufs=4) as p_w1,
            tc.tile_pool(name="small", bufs=28) as p_small,
            tc.tile_pool(name="state", bufs=1) as p_state,
        ):
            # ---------------- persistent state ----------------
            def T(shape, name, dt=F32):
                return p_state.tile(shape, dt, name=name, tag=name)

            hT = [T([128, HC, BL, L + 2], f"hT{i}", BF16) for i in range(2)]
            cT = [T([128, HC, BL, L + 2], f"cT{i}") for i in range(2)]
            gT = [T([128, HC, BL], f"gT{i}", BF16) for i in range(2)]
            cgT = [T([128, HC, BL], f"cgT{i}") for i in range(2)]
            xT = T([128, EC, N], "xT", F32R)
            gate_x = T([128, GC, N], "gate_x", F16)
            # gg rows 0:BL hold g @ [Wg_g | Wfi_g]; rows BL:128 stay zero so the
            # selector matmul can contract over a full K=128.
            gg_sb = T([128, GG_W], "gg_sb", F32R)
            x_nat = T([128, BL, EP], "x_nat")
            idx_sb = T([128, BL], "idx_sb", I32)
            mask_sb = T([128, N], "mask_sb")
            invlen_sb = T([128, BL], "invlen_sb")
            sel_sb = T([128, N], "sel_sb", F32R)
            ident_sb = T([128, 128], "ident_sb")
            wfi_sb = T([128, HC, H], "wfi_sb", BF16)
            w2_sb = T([128, 2 * HC, DP], "w2_sb", BF16)
            a1T = T([128, 2 * HC, BL], "a1T", BF16)
            bg_sb = T([128, GC], "bg_sb")
            b0_sb = T([128, HC], "b0_sb")
            bfi_sb = T([128, HC], "bfi_sb")
            bgf_sb = T([128, HC], "bgf_sb")
            bgo_sb = T([128, HC], "bgo_sb")
            b1_sb = T([128, 2 * HC], "b1_sb")
            b2_sb = T([BL, DP], "b2_sb")

            def mask3():
                return mask_sb[:].rearrange("p (e l) -> p e l", l=L)

            def v3(t):
                return t[:].rearrange("p (e l) -> p e l", l=L)

            def tmp2(name):
                return p_tmp.tile([128, N], F32, name=name, tag="tmp")

            def tmp3(name):
                return p_tmp.tile([128, BL, L], F32, name=name, tag="tmp")

            def sm(name):
                return p_small.tile([128, BL], F32, name=name, tag="sm")

            # ---------------- prologue: loads ----------------
            nc.sync.dma_start(idx_sb[:], tok_d.ap())
            nc.sync.dma_start(mask_sb[:], mask_d.ap())
            nc.sync.dma_start(invlen_sb[:], invlen_d.ap())
            nc.sync.dma_start(sel_sb[:], sel_d.ap())
            nc.sync.dma_start(ident_sb[:], ident_d.ap())
            nc.sync.dma_start(wfi_sb[:], wfi_h.ap())
            nc.sync.dma_start(w2_sb[:], w2_d.ap())
            for t_sb, t_d in (
                (bg_sb, bg_t), (b0_sb, b0_t), (bfi_sb, bfi_t),
                (bgf_sb, bgf_t), (bgo_sb, bgo_t), (b1_sb, b1_t), (b2_sb, b2_r),
            ):
                nc.sync.dma_start(t_sb[:], t_d.ap())

            # zero state (guard columns included)
            for t in (*hT, *gT):
                nc.vector.memset(t[:], 0.0)
            for t in (*cT, *cgT):
                nc.vector.memset(t[:], 0.0)
            nc.vector.memset(x_nat[:, :, E:], 0.0)  # pad cols only: gather writes [:E]
            nc.vector.memset(gg_sb[:].bitcast(F32), 0.0)

            # ---------------- prologue: embedding gather + transpose ----------------
            for e in range(BL):
                nc.gpsimd.indirect_dma_start(
                    out=x_nat[:, e, :E],
                    out_offset=None,
                    in_=embed_d.ap(),
                    in_offset=bass.IndirectOffsetOnAxis(ap=idx_sb[:, e : e + 1], axis=0),
                )
            for e in range(BL):
                for ec in range(EC):
                    pst = psumB.tile([128, 128], F32, name="pst", tag="pB")
                    nc.tensor.transpose(
                        pst[:], x_nat[:, e, ec * 128 : (ec + 1) * 128], ident_sb[:]
                    )
                    nc.scalar.copy(xT[:, ec, e * L : (e + 1) * L], pst[:])

            # ---------------- prologue: h0 = tanh(x@W0+b0)*mask, g0 ----------------
            for hk in range(HC):
                w0p = p_wx.tile([128, EC, 128], F32R, name="w0p", tag="wx")
                nc.sync.dma_start(w0p[:], w0_d.ap()[hk])
                ps = psumA.tile([128, N], F32, name="ps_h0", tag="pA")
                for kc in range(EC):
                    nc.tensor.matmul(
                        ps[:], w0p[:, kc], xT[:, kc],
                        start=(kc == 0), stop=(kc == EC - 1),
                    )
                h0t = tmp2("h0t")
                nc.scalar.activation(h0t[:], ps[:], AF.Tanh, bias=b0_sb[:, hk : hk + 1])
                nc.vector.tensor_mul(
                    out=hT[0][:, hk, :, 1 : L + 1], in0=v3(h0t), in1=mask3()
                )
                hsum = sm("hsum")
                nc.vector.reduce_sum(
                    hsum[:], hT[0][:, hk, :, 1 : L + 1], axis=AX.X
                )
                nc.vector.tensor_mul(out=gT[0][:, hk], in0=hsum[:], in1=invlen_sb[:])

            # ---------------- prologue: gate_x = x@Wg_x + bg ----------------
            for m in range(GC):
                wxp = p_wx.tile([128, EC, 128], F32R, name="wxp", tag="wx")
                nc.sync.dma_start(wxp[:], wg_x.ap()[m])
                ps = psumA.tile([128, N], F32, name="ps_gx", tag="pA")
                for kc in range(EC):
                    nc.tensor.matmul(
                        ps[:], wxp[:, kc], xT[:, kc],
                        start=(kc == 0), stop=(kc == EC - 1),
                    )
                nc.scalar.activation(
                    gate_x[:, m], ps[:], AF.Identity, bias=bg_sb[:, m : m + 1]
                )

            # ---------------- steps ----------------
            for s in range(NUM_STEPS):
                cur, nxt = s % 2, (s + 1) % 2
                h_c, h_n = hT[cur], hT[nxt]
                c_c, c_n = cT[cur], cT[nxt]
                g_c, g_n = gT[cur], gT[nxt]
                cg_c, cg_n = cgT[cur], cgT[nxt]

                def emit_gg(g_c=g_c):
                    # gg[0:BL] = g @ [Wg_g | Wfi_g]; column groups produced in
                    # the order the big-GEMM selectors consume them
                    for nj in (6, 0, 1, 2, 3, 4, 5, 7):
                        psg = psumB.tile([BL, 512], F32, name="psg", tag="pB")
                        for kc in range(HC):
                            wcp = p_wcat.tile([128, 512], BF16, name="wcp", tag="wc")
                            nc.sync.dma_start(wcp[:], wg_gcat.ap()[kc, nj])
                            nc.tensor.matmul(
                                psg[:], g_c[:, kc], wcp[:],
                                start=(kc == 0), stop=(kc == HC - 1),
                            )
                        nc.scalar.copy(gg_sb[0:BL, nj * 512 : (nj + 1) * 512], psg[:])

                def emit_hmm(m, h_c=h_c):
                    # 12 accumulating matmuls: hl / h / hr parts
                    wp = p_wg.tile([128, KHH, 128], BF16, name="wp", tag="wg")
                    nc.sync.dma_start(wp[:], wg_hhh.ap()[m])
                    ps = psumA.tile([128, N], F32, name="ps_g", tag="pA")
                    for kc in range(KHH):
                        off = kc // HC  # 0: h_{i-1}, 1: h_i, 2: h_{i+1}
                        q = kc % HC
                        nc.tensor.matmul(
                            ps[:], wp[:, kc], h_c[:, q, :, off : off + L],
                            start=(kc == 0), stop=False,
                        )
                    return ps

                def emit_sel_evict(m, ps, j):
                    nc.tensor.matmul(
                        ps[:], gg_sb[:, m * 128 : (m + 1) * 128], sel_sb[:],
                        start=False, stop=True,
                    )
                    gbuf = tmp2("gbuf")
                    nc.vector.scalar_tensor_tensor(
                        out=gbuf[:], in0=ps[:], scalar=1.0, in1=gate_x[:, m],
                        op0=AL.mult, op1=AL.add,
                    )
                    et = p_gate.tile([128, N], F32, name=f"eg{j}", tag="gate")
                    fn = AF.Exp if j < 5 else (AF.Sigmoid if j == 5 else AF.Tanh)
                    nc.scalar.activation(et[:], gbuf[:], fn)
                    return et

                J_ORDER = (6, 0, 1, 2, 3, 4, 5)  # u first, exps, o last
                h_avg = []
                for hk in range(HC):
                    eg = {}
                    for idx, j in enumerate(J_ORDER):
                        m = j * HC + hk
                        ps = emit_hmm(m)
                        if hk == 0 and idx == 0:
                            # gg GEMM goes here: its g_n dependency then hides
                            # under chunk 0's 12 independent matmuls.
                            emit_gg()
                        eg[j] = emit_sel_evict(m, ps, j)
                        # emit recurrence ops as soon as inputs exist, so the
                        # DVE/GpSimd streams never queue behind later evicts
                        if idx == 1:
                            m1 = tmp2("m1")
                            nc.vector.tensor_mul(m1[:], eg[0][:], eg[6][:])
                        elif idx == 2:
                            s01 = tmp2("s01")
                            nc.gpsimd.tensor_add(s01[:], eg[0][:], eg[1][:])
                            t1 = tmp3("t1")
                            nc.gpsimd.tensor_mul(t1[:], v3(eg[1]), c_c[:, hk, :, 0:L])
                        elif idx == 3:
                            t2 = tmp3("t2")
                            nc.gpsimd.tensor_mul(
                                t2[:], v3(eg[2]), c_c[:, hk, :, 1 : L + 1]
                            )
                        elif idx == 4:
                            s23 = tmp2("s23")
                            nc.gpsimd.tensor_add(s23[:], eg[2][:], eg[3][:])
                            t3 = tmp3("t3")
                            nc.gpsimd.tensor_mul(
                                t3[:], v3(eg[3]), c_c[:, hk, :, 2 : L + 2]
                            )
                            p12 = tmp2("p12")
                            nc.vector.tensor_add(p12[:], t1[:], t2[:])
                        elif idx == 5:
                            s03 = tmp2("s03")
                            nc.gpsimd.tensor_add(s03[:], s01[:], s23[:])
                            S5 = tmp2("S5")
                            nc.vector.tensor_add(S5[:], s03[:], eg[4][:])
                            r5 = tmp2("r5")
                            nc.vector.reciprocal_approx_fast(r5[:], S5[:])
                            rm = tmp2("rm")
                            nc.gpsimd.tensor_mul(rm[:], r5[:], mask_sb[:])
                            # p34 = t3 + Es*cg, fused per example
                            p34 = tmp3("p34")
                            es3 = v3(eg[4])
                            for e in range(BL):
                                nc.vector.scalar_tensor_tensor(
                                    out=p34[:, e], in0=es3[:, e],
                                    scalar=cg_c[:, hk, e : e + 1], in1=t3[:, e],
                                    op0=AL.mult, op1=AL.add,
                                )
                            pre = tmp2("pre")
                            nc.vector.tensor_add(pre[:], p12[:], p34[:])
                            acc = tmp2("acc")
                            nc.vector.tensor_add(acc[:], pre[:], m1[:])
                            nc.vector.tensor_mul(
                                out=c_n[:, hk, :, 1 : L + 1], in0=v3(acc), in1=v3(rm)
                            )
                            tanh_c = tmp3("tanh_c")
                            nc.scalar.activation(
                                tanh_c[:], c_n[:, hk, :, 1 : L + 1], AF.Tanh
                            )
                    # after the o gate: h_new and its average
                    nc.vector.tensor_mul(
                        out=h_n[:, hk, :, 1 : L + 1], in0=v3(eg[5]), in1=tanh_c[:]
                    )
                    hsum = sm("hsum2")
                    nc.vector.reduce_sum(hsum[:], h_n[:, hk, :, 1 : L + 1], axis=AX.X)
                    hav = p_small.tile([128, BL], BF16, name="hav", tag="sm")
                    nc.vector.tensor_mul(hav[:], hsum[:], invlen_sb[:])
                    h_avg.append(hav)

                # -- fi GEMM: kc=3 (last-written h chunk) deferred to the end of
                # each accumulation so PE has ready work while h_n[3] finishes
                efims = []
                psfs = []
                for hk in range(HC):
                    psf = psumA.tile([128, N], F32, name="psf", tag="pA")
                    for kc in range(HC - 1):
                        nc.tensor.matmul(
                            psf[:], wfi_sb[:, kc, hk * 128 : (hk + 1) * 128],
                            h_n[:, kc, :, 1 : L + 1],
                            start=(kc == 0), stop=False,
                        )
                    nc.tensor.matmul(
                        psf[:],
                        gg_sb[:, 7 * H + hk * 128 : 7 * H + (hk + 1) * 128],
                        sel_sb[:],
                        start=False, stop=False,
                    )
                    psfs.append(psf)
                # keep-warm punctuation: tiny throwaway matmuls that depend on
                # successive points of the hk=3 recurrence chain. They execute
                # spread across the tail wait, so the PE activity monitor never
                # sees a full idle window and the clock stays at 2.4 GHz.
                for dep in (r5[:, :128], acc[:, :128], tanh_c[:, 0]):
                    dmy = psumB.tile([64, 128], F32, name="dmy", tag="pB")
                    nc.tensor.matmul(
                        dmy[:, : dep.free_size()], mask_sb[:, :64], dep,
                        start=True, stop=True,
                    )
                for hk in range(HC):
                    psf = psfs[hk]
                    nc.tensor.matmul(
                        psf[:], wfi_sb[:, HC - 1, hk * 128 : (hk + 1) * 128],
                        h_n[:, HC - 1, :, 1 : L + 1],
                        start=False, stop=True,
                    )
                    efi = tmp2("efi")
                    nc.scalar.activation(
                        efi[:], psf[:], AF.Exp, bias=bfi_sb[:, hk : hk + 1]
                    )
                    efim = tmp2("efim")
                    nc.gpsimd.tensor_mul(efim[:], efi[:], mask_sb[:])
                    efims.append(efim)

                # -- fg / og GEMMs (transposed, N=4), in pairs with the
                # h_avg[3] contribution deferred to keep PE fed
                res_sm = {}
                for pair in range(HC):
                    mos = (2 * pair, 2 * pair + 1)
                    psts = []
                    for mo in mos:
                        w_d = wgf_d if mo < HC else wgo_d
                        mm = mo % HC
                        wfp = p_wgfgo.tile(
                            [128, 2 * HC, 128], BF16, name="wfp", tag="wf"
                        )
                        nc.sync.dma_start(wfp[:], w_d.ap()[mm])
                        pst = psumT.tile([128, BL], F32, name="pst_f", tag="pT")
                        for kc in range(2 * HC - 1):
                            rhs = g_c[:, kc] if kc < HC else h_avg[kc - HC][:]
                            nc.tensor.matmul(
                                pst[:], wfp[:, kc], rhs,
                                start=(kc == 0), stop=False,
                            )
                        psts.append((pst, wfp))
                    for mo, (pst, wfp) in zip(mos, psts):
                        mm = mo % HC
                        nc.tensor.matmul(
                            pst[:], wfp[:, 2 * HC - 1], h_avg[HC - 1][:],
                            start=False, stop=True,
                        )
                        r_sm = sm("r_sm")
                        if mo < HC:
                            nc.scalar.activation(
                                r_sm[:], pst[:], AF.Exp, bias=bgf_sb[:, mm : mm + 1]
                            )
                        else:
                            nc.scalar.activation(
                                r_sm[:], pst[:], AF.Sigmoid,
                                bias=bgo_sb[:, mm : mm + 1],
                            )
                        res_sm[mo] = r_sm
                efg = [res_sm[i] for i in range(HC)]
                ogs = [res_sm[HC + i] for i in range(HC)]

                # -- slot softmax + cg/g update
                for hk in range(HC):
                    efim = efims[hk]
                    pw = tmp3("pw")
                    nc.vector.tensor_mul(pw[:], v3(efim), c_n[:, hk, :, 1 : L + 1])
                    s_c = sm("s_c")
                    nc.vector.reduce_sum(s_c[:], pw[:], axis=AX.X)
                    ssum = sm("ssum")
                    nc.vector.reduce_sum(ssum[:], v3(efim), axis=AX.X)
                    den = sm("den")
                    nc.vector.tensor_add(den[:], efg[hk][:], ssum[:])
                    rden = sm("rden")
                    nc.vector.reciprocal(rden[:], den[:])
                    tnum = sm("tnum")
                    nc.vector.tensor_mul(tnum[:], efg[hk][:], cg_c[:, hk])
                    num = sm("num")
                    nc.vector.tensor_add(num[:], tnum[:], s_c[:])
                    nc.vector.tensor_mul(out=cg_n[:, hk], in0=num[:], in1=rden[:])
                    tcg = sm("tcg")
                    nc.scalar.activation(tcg[:], cg_n[:, hk], AF.Tanh)
                    nc.vector.tensor_mul(out=g_n[:, hk], in0=ogs[hk][:], in1=tcg[:])

            # ---------------- epilogue ----------------
            g_fin = gT[NUM_STEPS % 2]
            for mo in range(2 * HC):
                w1p = p_w1.tile([128, HC, 128], BF16, name="w1p", tag="w1")
                nc.sync.dma_start(w1p[:], w1_d.ap()[mo])
                pst = psumT.tile([128, BL], F32, name="pst_a1", tag="pT")
                for kc in range(HC):
                    nc.tensor.matmul(
                        pst[:], w1p[:, kc], g_fin[:, kc],
                        start=(kc == 0), stop=(kc == HC - 1),
                    )
                nc.scalar.activation(
                    a1T[:, mo], pst[:], AF.Tanh, bias=b1_sb[:, mo : mo + 1]
                )

            pslg = psumB.tile([BL, DP], F32, name="pslg", tag="pB")
            for kc in range(2 * HC):
                nc.tensor.matmul(
                    pslg[:], a1T[:, kc], w2_sb[:, kc],
                    start=(kc == 0), stop=(kc == 2 * HC - 1),
                )
            lg = p_small.tile([BL, DP], F32, name="lg", tag="lg")
            nc.vector.tensor_add(lg[:], pslg[:], b2_sb[:])
            mx = p_small.tile([BL, 1], F32, name="mx", tag="lg")
            nc.vector.reduce_max(mx[:], lg[:, :DOUT], axis=AX.X)
            tsh = p_small.tile([BL, DOUT], F32, name="tsh", tag="lg")
            nc.vector.tensor_scalar(tsh[:], lg[:, :DOUT], mx[:], None, AL.subtract)
            ex = p_small.tile([BL, DOUT], F32, name="ex", tag="lg")
            ssum = p_small.tile([BL, 1], F32, name="ssum_l", tag="lg")
            nc.scalar.activation(ex[:], tsh[:], AF.Exp, accum_out=ssum[:])
            lse = p_small.tile([BL, 1], F32, name="lse", tag="lg")
            nc.scalar.activation(lse[:], ssum[:], AF.Ln)
            res = p_small.tile([BL, DOUT], F32, name="res", tag="lg")
            nc.vector.tensor_scalar(res[:], tsh[:], lse[:], None, AL.subtract)
            nc.sync.dma_start(out_d.ap(), res[:])

    nc.compile()
    return nc


def prep_in_maps(inputs):
    """Host-side prep: slice per core, pad/retile weights. Returns in_maps."""
    tokens = np.asarray(inputs["tokens"]).astype(np.int32)
    lengths = np.asarray(inputs["lengths"]).astype(np.int32)
    f = lambda k: np.ascontiguousarray(np.asarray(inputs[k], dtype=np.float32))
    embed = f("embed")
    W0, b0 = f("W0"), f("b0")
    Wg, bg = f("Wg"), f("bg")
    Wgf, bgf = f("Wgf"), f("bgf")
    Wfi, bfi = f("Wfi"), f("bfi")
    Wgo, bgo = f("Wgo"), f("bgo")
    W1, b1 = f("W1"), f("b1")
    W2, b2 = f("W2"), f("b2")

    def tile_km(w, kc, mc):
        # [kc*128, mc*128] -> [mc, 128, kc, 128]: piece[m][p,k,c] = w[k*128+p, m*128+c]
        return np.ascontiguousarray(
            w.reshape(kc, 128, mc, 128).transpose(2, 1, 0, 3)
        )

    wg_hhh = tile_km(Wg[: 3 * H], KHH, GC)
    wg_x_pad = np.zeros((EP, 7 * H), np.float32)
    wg_x_pad[:E] = Wg[3 * H : 3 * H + E]
    wg_x = tile_km(wg_x_pad, EC, GC)
    gcat = np.concatenate([Wg[3 * H + E :], Wfi[:H]], axis=1)  # [512, 4096]
    wg_gcat = np.ascontiguousarray(
        gcat.reshape(HC, 128, GGC, 512).transpose(0, 2, 1, 3)
    )
    wfi_hp = np.ascontiguousarray(Wfi[H:].reshape(HC, 128, H).transpose(1, 0, 2))
    wgf = tile_km(Wgf, 2 * HC, HC)
    wgo = tile_km(Wgo, 2 * HC, HC)
    w0_pad = np.zeros((EP, H), np.float32)
    w0_pad[:E] = W0
    w0 = tile_km(w0_pad, EC, HC)
    w1 = tile_km(W1, HC, 2 * HC)
    w2p = np.zeros((2 * H, DP), np.float32)
    w2p[:, :DOUT] = W2
    b2p = np.zeros((DP,), np.float32)
    b2p[:DOUT] = b2
    w2 = np.ascontiguousarray(w2p.reshape(2 * HC, 128, DP).transpose(1, 0, 2))

    def t_bias(b):
        return np.ascontiguousarray(b.reshape(-1, 128).T)

    sel = np.zeros((128, N), np.float32)
    for e in range(BL):
        sel[e, e * L : (e + 1) * L] = 1.0
    ident = np.eye(128, dtype=np.float32)

    bf = ml_dtypes.bfloat16
    shared = dict(
        embed=embed, wg_hhh=wg_hhh.astype(bf), wg_x=wg_x,
        wg_gcat=wg_gcat.astype(bf), wfi_h=wfi_hp.astype(bf),
        wgf=wgf.astype(bf), wgo=wgo.astype(bf), w0=w0,
        w1=w1.astype(bf), w2=w2.astype(bf),
        bg_t=t_bias(bg), b0_t=t_bias(b0), bfi_t=t_bias(bfi), bgf_t=t_bias(bgf),
        bgo_t=t_bias(bgo), b1_t=t_bias(b1),
        b2_r=np.ascontiguousarray(np.tile(b2p[None, :], (BL, 1))),
        sel=sel, ident=ident,
    )

    in_maps = []
    for c in range(NCORES):
        sl = slice(c * BL, (c + 1) * BL)
        tok = tokens[sl]                                   # [BL, L]
        lens = np.maximum(lengths[sl].astype(np.float32), 1.0)
        mask = (np.arange(L)[None, :] < lengths[sl][:, None]).astype(np.float32)
        mask_rep = np.ascontiguousarray(
            np.broadcast_to(mask.reshape(1, N), (128, N))
        )
        invlen_rep = np.ascontiguousarray(
            np.broadcast_to((1.0 / lens).reshape(1, BL), (128, BL))
        )
        tok_idx = np.ascontiguousarray(tok.T.astype(np.int32))  # [L=128, BL]
        m = dict(shared)
        m.update(tok_idx=tok_idx, mask_rep=mask_rep, invlen_rep=invlen_rep)
        in_maps.append(m)
    return in_maps


_NC_CACHE = {}


def kernel(**inputs) -> np.ndarray:
    in_maps = prep_in_maps(inputs)
    if "nc" not in _NC_CACHE:
        _NC_CACHE["nc"] = build_nc()
    nc = _NC_CACHE["nc"]
    res = run_bass_kernel_spmd(nc, in_maps, core_ids=list(range(NCORES)))
    return np.concatenate([r["out"] for r in res.results], axis=0)


if __name__ == "__main__":
    nc = build_nc()
    print("built ok")



# revision 22
# speedup vs baseline: 1.0155x; 1.0155x over previous
"""Trainium2 Bass kernel for the S-LSTM (sentence-state LSTM) classifier.

Data-parallel over batch: 8 cores x 4 examples. Everything on-chip runs in a
"transposed" layout: feature channels on SBUF partitions, (example, position)
flattened on the free dim (4*128 = 512 columns). The per-step gate GEMM
computes gates.T = Wg.T @ ctx.T with Wg slices stationary and h.T moving at
N=512 (fp32 data in float32r mode -> full PE rate). Position shifts
(h_{i-1}, h_{i+1}, c shifts) are free-dim offsets into state tiles that carry
one zero guard column on each side of every example's 128 columns.

The global-node ("g") part of ctx is rank-1 along positions: gg = g @ Wg_g is
computed once per step as a tiny M=4 GEMM, then folded into the big GEMM as an
extra K chunk against a constant 0/1 selector matrix (zero-padded to K=128).

Weights are streamed from HBM each step in host-pre-tiled contiguous pieces
(one DMA per stationary block column), overlapped with PE work.
"""

import ml_dtypes
import numpy as np

import concourse.bass as bass
import concourse.mybir as mybir
from concourse import bacc
import concourse.tile as tile
from concourse.bass_utils import run_bass_kernel_spmd

F32 = mybir.dt.float32
F32R = mybir.dt.float32r
F16 = mybir.dt.float16
BF16 = mybir.dt.bfloat16
FP8 = mybir.dt.float8e4
I32 = mybir.dt.int32
AL = mybir.AluOpType
AF = mybir.ActivationFunctionType
AX = mybir.AxisListType
DR = mybir.MatmulPerfMode.DoubleRow

B, L, V, E, H, DOUT = 32, 128, 30000, 300, 512, 5
NUM_STEPS = 5
NCORES = 8
BL = B // NCORES          # 4 examples per core
N = BL * L                # 512 free columns
EP = 384                  # E padded to 3*128
HC = H // 128             # 4 H chunks
GC = 7 * H // 128         # 28 gate output chunks
KHH = 3 * HC              # 12 K chunks for hl/h/hr
EC = EP // 128            # 3 E chunks
EP4 = 512                 # E padded to 4*128 (fp8 DoubleRow pairs)
EC4 = EP4 // 128          # 4 E chunks for the fp8 gate_x GEMM
GG_W = 7 * H + H          # 4096: [Wg_g | Wfi_g] columns
GGC = GG_W // 512         # 8
DP = 8                    # DOUT padded to even size for fp32r matmul

# fp8 scales (powers of two; descales fold into existing evict scale params)
SH = 16.0                 # h state scale (|h| <= 1 -> <= 16)
SW = 2048.0               # Wg/Wfi weight scale (absmax ~0.11 -> <= 222 < 240)
SX = 256.0                # embedding scale (absmax ~0.52 -> <= 134)
SWX = 2048.0              # Wg_x weight scale
DS_HW = 1.0 / (SH * SW)   # descale for h-side fp8 GEMMs
DS_XW = 1.0 / (SX * SWX)  # descale for the x-side fp8 GEMM


def build_nc():
    nc = bacc.Bacc(trn_type="TRN2", target_bir_lowering=False)

    d = {}

    def din(name, shape, dt=F32):
        d[name] = nc.dram_tensor(name, list(shape), dt, kind="ExternalInput")
        return d[name]

    # weights are host-pre-tiled so every DMA reads contiguous HBM
    embed_d = din("embed", (V, E))
    wg_hhh = din("wg_hhh", (GC, 128, KHH, 128), FP8)
    wg_x = din("wg_x", (GC, 128, EC4, 128), FP8)
    wg_gcat = din("wg_gcat", (HC, GGC, 128, 512), BF16)
    wfi_h = din("wfi_h", (128, HC, H), FP8)
    wgf_d = din("wgf", (HC, 128, 2 * HC, 128), BF16)
    wgo_d = din("wgo", (HC, 128, 2 * HC, 128), BF16)
    w0_d = din("w0", (HC, 128, EC, 128), F32R)
    w1_d = din("w1", (2 * HC, 128, HC, 128), BF16)
    w2_d = din("w2", (128, 2 * HC, DP), BF16)
    bg_t = din("bg_t", (128, GC))
    b0_t = din("b0_t", (128, HC))
    bfi_t = din("bfi_t", (128, HC))
    bgf_t = din("bgf_t", (128, HC))
    bgo_t = din("bgo_t", (128, HC))
    b1_t = din("b1_t", (128, 2 * HC))
    b2_r = din("b2_r", (BL, DP))
    sel_d = din("sel", (128, N), F32R)
    ident_d = din("ident", (128, 128))
    tok_d = din("tok_idx", (128, BL), I32)       # column e = tokens of example e
    mask_d = din("mask_rep", (128, N))
    invlen_d = din("invlen_rep", (128, BL))

    out_d = nc.dram_tensor("out", [BL, DOUT], F32, kind="ExternalOutput")

    with tile.TileContext(nc) as tc:
        with (
            tc.tile_pool(name="psumA", bufs=4, space="PSUM") as psumA,
            tc.tile_pool(name="psumB", bufs=2, space="PSUM") as psumB,
            tc.tile_pool(name="psumT", bufs=2, space="PSUM") as psumT,
            tc.tile_pool(name="gates", bufs=10) as p_gate,
            tc.tile_pool(name="tmp", bufs=12) as p_tmp,
            tc.tile_pool(name="wg", bufs=6) as p_wg,
            tc.tile_pool(name="wcat", bufs=6) as p_wcat,
            tc.tile_pool(name="wgfgo", bufs=4) as p_wgfgo,
            tc.tile_pool(name="wx", bufs=3) as p_wx,
            tc.tile_pool(name="w1p", bufs=4) as p_w1,
            tc.tile_pool(name="small", bufs=28) as p_small,
            tc.tile_pool(name="state", bufs=1) as p_state,
        ):
            # ---------------- persistent state ----------------
            def T(shape, name, dt=F32):
                return p_state.tile(shape, dt, name=name, tag=name)

            # h is stored densely (no guard cols) so DoubleRow rhs views are
            # 3-D [128, 2, N]; the +-1 position shifts are materialized into
            # hL/hR by SBUF->SBUF DMAs (boundary zero cols written once).
            hD = [T([128, HC, N], f"hD{i}", FP8) for i in range(2)]
            hL = [T([128, HC, N], f"hL{i}", FP8) for i in range(2)]
            hR = [T([128, HC, N], f"hR{i}", FP8) for i in range(2)]
            cT = [T([128, HC, BL, L + 2], f"cT{i}") for i in range(2)]
            gT = [T([128, HC, BL], f"gT{i}", BF16) for i in range(2)]
            cgT = [T([128, HC, BL], f"cgT{i}") for i in range(2)]
            xT = T([128, EC, N], "xT", F32R)
            xT8 = T([128, EC4, N], "xT8", FP8)
            gate_x = T([128, GC, N], "gate_x", F16)
            # gg rows 0:BL hold g @ [Wg_g | Wfi_g]; rows BL:128 stay zero so the
            # selector matmul can contract over a full K=128.
            gg_sb = T([128, GG_W], "gg_sb", F32R)
            x_nat = T([128, BL, EP], "x_nat")
            idx_sb = T([128, BL], "idx_sb", I32)
            mask_sb = T([128, N], "mask_sb")
            invlen_sb = T([128, BL], "invlen_sb")
            sel_sb = T([128, N], "sel_sb", F32R)
            ident_sb = T([128, 128], "ident_sb")
            wfi_sb = T([128, HC, H], "wfi_sb", FP8)
            w2_sb = T([128, 2 * HC, DP], "w2_sb", BF16)
            a1T = T([128, 2 * HC, BL], "a1T", BF16)
            bg_sb = T([128, GC], "bg_sb")
            b0_sb = T([128, HC], "b0_sb")
            bfi_sb = T([128, HC], "bfi_sb")
            bgf_sb = T([128, HC], "bgf_sb")
            bgo_sb = T([128, HC], "bgo_sb")
            b1_sb = T([128, 2 * HC], "b1_sb")
            b2_sb = T([BL, DP], "b2_sb")

            def mask3():
                return mask_sb[:].rearrange("p (e l) -> p e l", l=L)

            def v3(t):
                return t[:].rearrange("p (e l) -> p e l", l=L)

            def v3a(ap):
                return ap.rearrange("p (e l) -> p e l", l=L)

            def emit_shift_dmas(hd, hl, hr, hk):
                # hl[i] = h[i-1], hr[i] = h[i+1]; per-example boundary cols
                # stay zero from the prologue memset.
                d3 = v3a(hd[:, hk])
                nc.sync.dma_start(v3a(hl[:, hk])[:, :, 1:L], d3[:, :, 0 : L - 1])
                nc.sync.dma_start(v3a(hr[:, hk])[:, :, 0 : L - 1], d3[:, :, 1:L])

            def tmp2(name):
                return p_tmp.tile([128, N], F32, name=name, tag="tmp")

            def tmp3(name):
                return p_tmp.tile([128, BL, L], F32, name=name, tag="tmp")

            def sm(name):
                return p_small.tile([128, BL], F32, name=name, tag="sm")

            # ---------------- prologue: loads ----------------
            nc.sync.dma_start(idx_sb[:], tok_d.ap())
            nc.sync.dma_start(mask_sb[:], mask_d.ap())
            nc.sync.dma_start(invlen_sb[:], invlen_d.ap())
            nc.sync.dma_start(sel_sb[:], sel_d.ap())
            nc.sync.dma_start(ident_sb[:], ident_d.ap())
            nc.sync.dma_start(wfi_sb[:], wfi_h.ap())
            nc.sync.dma_start(w2_sb[:], w2_d.ap())
            for t_sb, t_d in (
                (bg_sb, bg_t), (b0_sb, b0_t), (bfi_sb, bfi_t),
                (bgf_sb, bgf_t), (bgo_sb, bgo_t), (b1_sb, b1_t), (b2_sb, b2_r),
            ):
                nc.sync.dma_start(t_sb[:], t_d.ap())

            # zero state (boundary columns of hL/hR included, written once)
            for t in (*hD, *hL, *hR, *gT):
                nc.vector.memset(t[:], 0.0)
            for t in (*cT, *cgT):
                nc.vector.memset(t[:], 0.0)
            nc.vector.memset(x_nat[:, :, E:], 0.0)  # pad cols only: gather writes [:E]
            nc.vector.memset(gg_sb[:].bitcast(F32), 0.0)
            nc.vector.memset(xT8[:, EC:], 0.0)  # fp8 pad chunk (rows 384:512)

            # ---------------- prologue: embedding gather + transpose ----------------
            for e in range(BL):
                nc.gpsimd.indirect_dma_start(
                    out=x_nat[:, e, :E],
                    out_offset=None,
                    in_=embed_d.ap(),
                    in_offset=bass.IndirectOffsetOnAxis(ap=idx_sb[:, e : e + 1], axis=0),
                )
            for e in range(BL):
                for ec in range(EC):
                    pst = psumB.tile([128, 128], F32, name="pst", tag="pB")
                    nc.tensor.transpose(
                        pst[:], x_nat[:, e, ec * 128 : (ec + 1) * 128], ident_sb[:]
                    )
                    nc.scalar.copy(xT[:, ec, e * L : (e + 1) * L], pst[:])
                    nc.scalar.activation(
                        xT8[:, ec, e * L : (e + 1) * L], pst[:], AF.Identity, scale=SX
                    )

            # ---------------- prologue: h0 = tanh(x@W0+b0)*mask, g0 ----------------
            for hk in range(HC):
                w0p = p_wx.tile([128, EC, 128], F32R, name="w0p", tag="wx")
                nc.sync.dma_start(w0p[:], w0_d.ap()[hk])
                ps = psumA.tile([128, N], F32, name="ps_h0", tag="pA")
                for kc in range(EC):
                    nc.tensor.matmul(
                        ps[:], w0p[:, kc], xT[:, kc],
                        start=(kc == 0), stop=(kc == EC - 1),
                    )
                h0t = tmp2("h0t")
                nc.scalar.activation(h0t[:], ps[:], AF.Tanh, bias=b0_sb[:, hk : hk + 1])
                nc.vector.scalar_tensor_tensor(
                    out=hD[0][:, hk], in0=h0t[:], scalar=SH,
                    in1=mask_sb[:], op0=AL.mult, op1=AL.mult,
                )
                emit_shift_dmas(hD[0], hL[0], hR[0], hk)
                hsum = sm("hsum")
                nc.vector.reduce_sum(hsum[:], v3a(hD[0][:, hk]), axis=AX.X)
                nc.vector.scalar_tensor_tensor(
                    out=gT[0][:, hk], in0=hsum[:], scalar=1.0 / SH,
                    in1=invlen_sb[:], op0=AL.mult, op1=AL.mult,
                )

            # ---------------- prologue: gate_x = x@Wg_x + bg ----------------
            for m in range(GC):
                wxp = p_wx.tile([128, EC4, 128], FP8, name="wxp", tag="wx")
                nc.sync.dma_start(wxp[:], wg_x.ap()[m])
                ps = psumA.tile([128, N], F32, name="ps_gx", tag="pA")
                for kp in range(EC4 // 2):
                    nc.tensor.matmul(
                        ps[:], wxp[:, 2 * kp : 2 * kp + 2],
                        xT8[:, 2 * kp : 2 * kp + 2],
                        start=(kp == 0), stop=(kp == EC4 // 2 - 1), perf_mode=DR,
                    )
                nc.scalar.activation(
                    gate_x[:, m], ps[:], AF.Identity, bias=bg_sb[:, m : m + 1],
                    scale=DS_XW,
                )

            # ---------------- steps ----------------
            for s in range(NUM_STEPS):
                cur, nxt = s % 2, (s + 1) % 2
                h_c, h_n = hD[cur], hD[nxt]
                hl_c, hr_c = hL[cur], hR[cur]
                hl_n, hr_n = hL[nxt], hR[nxt]
                c_c, c_n = cT[cur], cT[nxt]
                g_c, g_n = gT[cur], gT[nxt]
                cg_c, cg_n = cgT[cur], cgT[nxt]

                def emit_gg(g_c=g_c):
                    # gg[0:BL] = g @ [Wg_g | Wfi_g]; column groups produced in
                    # the order the big-GEMM selectors consume them
                    for nj in (6, 0, 1, 2, 3, 4, 5, 7):
                        psg = psumB.tile([BL, 512], F32, name="psg", tag="pB")
                        for kc in range(HC):
                            wcp = p_wcat.tile([128, 512], BF16, name="wcp", tag="wc")
                            nc.sync.dma_start(wcp[:], wg_gcat.ap()[kc, nj])
                            nc.tensor.matmul(
                                psg[:], g_c[:, kc], wcp[:],
                                start=(kc == 0), stop=(kc == HC - 1),
                            )
                        nc.scalar.copy(gg_sb[0:BL, nj * 512 : (nj + 1) * 512], psg[:])

                def emit_hmm(m, shift_tiles=(hl_c, h_c, hr_c)):
                    # 6 DoubleRow fp8 matmuls: hl / h / hr parts (256 K-rows each)
                    wp = p_wg.tile([128, KHH, 128], FP8, name="wp", tag="wg")
                    nc.sync.dma_start(wp[:], wg_hhh.ap()[m])
                    ps = psumA.tile([128, N], F32, name="ps_g", tag="pA")
                    for kp in range(KHH // 2):
                        t = shift_tiles[kp // 2]  # 0: h_{i-1}, 1: h_i, 2: h_{i+1}
                        q = (kp % 2) * 2
                        nc.tensor.matmul(
                            ps[:], wp[:, 2 * kp : 2 * kp + 2], t[:, q : q + 2],
                            start=(kp == 0), stop=False, perf_mode=DR,
                        )
                    return ps

                def emit_sel_evict(m, ps, j):
                    nc.tensor.matmul(
                        ps[:], gg_sb[:, m * 128 : (m + 1) * 128], sel_sb[:],
                        start=False, stop=True,
                    )
                    gbuf = tmp2("gbuf")
                    nc.vector.scalar_tensor_tensor(
                        out=gbuf[:], in0=ps[:], scalar=DS_HW, in1=gate_x[:, m],
                        op0=AL.mult, op1=AL.add,
                    )
                    et = p_gate.tile([128, N], F32, name=f"eg{j}", tag="gate")
                    fn = AF.Exp if j < 5 else (AF.Sigmoid if j == 5 else AF.Tanh)
                    nc.scalar.activation(et[:], gbuf[:], fn)
                    return et

                J_ORDER = (6, 0, 1, 2, 3, 4, 5)  # u first, exps, o last
                h_avg = []
                for hk in range(HC):
                    eg = {}
                    for idx, j in enumerate(J_ORDER):
                        m = j * HC + hk
                        ps = emit_hmm(m)
                        if hk == 0 and idx == 0:
                            # gg GEMM goes here: its g_n dependency then hides
                            # under chunk 0's 12 independent matmuls.
                            emit_gg()
                        eg[j] = emit_sel_evict(m, ps, j)
                        # emit recurrence ops as soon as inputs exist, so the
                        # DVE/GpSimd streams never queue behind later evicts
                        if idx == 1:
                            m1 = tmp2("m1")
                            nc.vector.tensor_mul(m1[:], eg[0][:], eg[6][:])
                        elif idx == 2:
                            s01 = tmp2("s01")
                            nc.gpsimd.tensor_add(s01[:], eg[0][:], eg[1][:])
                            t1 = tmp3("t1")
                            nc.gpsimd.tensor_mul(t1[:], v3(eg[1]), c_c[:, hk, :, 0:L])
                        elif idx == 3:
                            t2 = tmp3("t2")
                            nc.gpsimd.tensor_mul(
                                t2[:], v3(eg[2]), c_c[:, hk, :, 1 : L + 1]
                            )
                        elif idx == 4:
                            s23 = tmp2("s23")
                            nc.gpsimd.tensor_add(s23[:], eg[2][:], eg[3][:])
                            t3 = tmp3("t3")
                            nc.gpsimd.tensor_mul(
                                t3[:], v3(eg[3]), c_c[:, hk, :, 2 : L + 2]
                            )
                            p12 = tmp2("p12")
                            nc.vector.tensor_add(p12[:], t1[:], t2[:])
                        elif idx == 5:
                            s03 = tmp2("s03")
                            nc.gpsimd.tensor_add(s03[:], s01[:], s23[:])
                            S5 = tmp2("S5")
                            nc.vector.tensor_add(S5[:], s03[:], eg[4][:])
                            r5 = tmp2("r5")
                            nc.vector.reciprocal_approx_fast(r5[:], S5[:])
                            rm = tmp2("rm")
                            nc.gpsimd.tensor_mul(rm[:], r5[:], mask_sb[:])
                            # p34 = t3 + Es*cg, fused per example
                            p34 = tmp3("p34")
                            es3 = v3(eg[4])
                            for e in range(BL):
                                nc.vector.scalar_tensor_tensor(
                                    out=p34[:, e], in0=es3[:, e],
                                    scalar=cg_c[:, hk, e : e + 1], in1=t3[:, e],
                                    op0=AL.mult, op1=AL.add,
                                )
                            pre = tmp2("pre")
                            nc.vector.tensor_add(pre[:], p12[:], p34[:])
                            acc = tmp2("acc")
                            nc.vector.tensor_add(acc[:], pre[:], m1[:])
                            nc.vector.tensor_mul(
                                out=c_n[:, hk, :, 1 : L + 1], in0=v3(acc), in1=v3(rm)
                            )
                            tanh_c = tmp3("tanh_c")
                            nc.scalar.activation(
                                tanh_c[:], c_n[:, hk, :, 1 : L + 1], AF.Tanh
                            )
                    # after the o gate: h_new (fp8, scaled by SH) and its average
                    nc.vector.scalar_tensor_tensor(
                        out=v3a(h_n[:, hk]), in0=v3(eg[5]), scalar=SH,
                        in1=tanh_c[:], op0=AL.mult, op1=AL.mult,
                    )
                    emit_shift_dmas(h_n, hl_n, hr_n, hk)
                    hsum = sm("hsum2")
                    nc.vector.reduce_sum(hsum[:], v3a(h_n[:, hk]), axis=AX.X)
                    hav = p_small.tile([128, BL], BF16, name="hav", tag="sm")
                    nc.vector.scalar_tensor_tensor(
                        out=hav[:], in0=hsum[:], scalar=1.0 / SH,
                        in1=invlen_sb[:], op0=AL.mult, op1=AL.mult,
                    )
                    h_avg.append(hav)

                # -- fi GEMM: kc=3 (last-written h chunk) deferred to the end of
                # each accumulation so PE has ready work while h_n[3] finishes
                efims = []
                psfs = []
                for hk in range(HC):
                    psf = psumA.tile([128, N], F32, name="psf", tag="pA")
                    nc.tensor.matmul(
                        psf[:], wfi_sb[:, 0:2, hk * 128 : (hk + 1) * 128],
                        h_n[:, 0:2],
                        start=True, stop=False, perf_mode=DR,
                    )
                    nc.tensor.matmul(
                        psf[:],
                        gg_sb[:, 7 * H + hk * 128 : 7 * H + (hk + 1) * 128],
                        sel_sb[:],
                        start=False, stop=False,
                    )
                    psfs.append(psf)
                # keep-warm punctuation: tiny throwaway matmuls that depend on
                # successive points of the hk=3 recurrence chain. They execute
                # spread across the tail wait, so the PE activity monitor never
                # sees a full idle window and the clock stays at 2.4 GHz.
                for dep in (r5[:, :128], acc[:, :128], tanh_c[:, 0]):
                    dmy = psumB.tile([64, 128], F32, name="dmy", tag="pB")
                    nc.tensor.matmul(
                        dmy[:, : dep.free_size()], mask_sb[:, :64], dep,
                        start=True, stop=True,
                    )
                for hk in range(HC):
                    psf = psfs[hk]
                    nc.tensor.matmul(
                        psf[:], wfi_sb[:, 2:4, hk * 128 : (hk + 1) * 128],
                        h_n[:, 2:4],
                        start=False, stop=True, perf_mode=DR,
                    )
                    efi = tmp2("efi")
                    nc.scalar.activation(
                        efi[:], psf[:], AF.Exp, bias=bfi_sb[:, hk : hk + 1],
                        scale=DS_HW,
                    )
                    efim = tmp2("efim")
                    nc.gpsimd.tensor_mul(efim[:], efi[:], mask_sb[:])
                    efims.append(efim)

                # -- fg / og GEMMs (transposed, N=4), in pairs with the
                # h_avg[3] contribution deferred to keep PE fed
                res_sm = {}
                for pair in range(HC):
                    mos = (2 * pair, 2 * pair + 1)
                    psts = []
                    for mo in mos:
                        w_d = wgf_d if mo < HC else wgo_d
                        mm = mo % HC
                        wfp = p_wgfgo.tile(
                            [128, 2 * HC, 128], BF16, name="wfp", tag="wf"
                        )
                        nc.sync.dma_start(wfp[:], w_d.ap()[mm])
                        pst = psumT.tile([128, BL], F32, name="pst_f", tag="pT")
                        for kc in range(2 * HC - 1):
                            rhs = g_c[:, kc] if kc < HC else h_avg[kc - HC][:]
                            nc.tensor.matmul(
                                pst[:], wfp[:, kc], rhs,
                                start=(kc == 0), stop=False,
                            )
                        psts.append((pst, wfp))
                    for mo, (pst, wfp) in zip(mos, psts):
                        mm = mo % HC
                        nc.tensor.matmul(
                            pst[:], wfp[:, 2 * HC - 1], h_avg[HC - 1][:],
                            start=False, stop=True,
                        )
                        r_sm = sm("r_sm")
                        if mo < HC:
                            nc.scalar.activation(
                                r_sm[:], pst[:], AF.Exp, bias=bgf_sb[:, mm : mm + 1]
                            )
                        else:
                            nc.scalar.activation(
                                r_sm[:], pst[:], AF.Sigmoid,
                                bias=bgo_sb[:, mm : mm + 1],
                            )
                        res_sm[mo] = r_sm
                efg = [res_sm[i] for i in range(HC)]
                ogs = [res_sm[HC + i] for i in range(HC)]

                # -- slot softmax + cg/g update
                for hk in range(HC):
                    efim = efims[hk]
                    pw = tmp3("pw")
                    nc.vector.tensor_mul(pw[:], v3(efim), c_n[:, hk, :, 1 : L + 1])
                    s_c = sm("s_c")
                    nc.vector.reduce_sum(s_c[:], pw[:], axis=AX.X)
                    ssum = sm("ssum")
                    nc.vector.reduce_sum(ssum[:], v3(efim), axis=AX.X)
                    den = sm("den")
                    nc.vector.tensor_add(den[:], efg[hk][:], ssum[:])
                    rden = sm("rden")
                    nc.vector.reciprocal(rden[:], den[:])
                    tnum = sm("tnum")
                    nc.vector.tensor_mul(tnum[:], efg[hk][:], cg_c[:, hk])
                    num = sm("num")
                    nc.vector.tensor_add(num[:], tnum[:], s_c[:])
                    nc.vector.tensor_mul(out=cg_n[:, hk], in0=num[:], in1=rden[:])
                    tcg = sm("tcg")
                    nc.scalar.activation(tcg[:], cg_n[:, hk], AF.Tanh)
                    nc.vector.tensor_mul(out=g_n[:, hk], in0=ogs[hk][:], in1=tcg[:])

            # ---------------- epilogue ----------------
            g_fin = gT[NUM_STEPS % 2]
            for mo in range(2 * HC):
                w1p = p_w1.tile([128, HC, 128], BF16, name="w1p", tag="w1")
                nc.sync.dma_start(w1p[:], w1_d.ap()[mo])
                pst = psumT.tile([128, BL], F32, name="pst_a1", tag="pT")
                for kc in range(HC):
                    nc.tensor.matmul(
                        pst[:], w1p[:, kc], g_fin[:, kc],
                        start=(kc == 0), stop=(kc == HC - 1),
                    )
                nc.scalar.activation(
                    a1T[:, mo], pst[:], AF.Tanh, bias=b1_sb[:, mo : mo + 1]
                )

            pslg = psumB.tile([BL, DP], F32, name="pslg", tag="pB")
            for kc in range(2 * HC):
                nc.tensor.matmul(
                    pslg[:], a1T[:, kc], w2_sb[:, kc],
                    start=(kc == 0), stop=(kc == 2 * HC - 1),
                )
            lg = p_small.tile([BL, DP], F32, name="lg", tag="lg")
            nc.vector.tensor_add(lg[:], pslg[:], b2_sb[:])
            mx = p_small.tile([BL, 1], F32, name="mx", tag="lg")
            nc.vector.reduce_max(mx[:], lg[:, :DOUT], axis=AX.X)
            tsh = p_small.tile([BL, DOUT], F32, name="tsh", tag="lg")
            nc.vector.tensor_scalar(tsh[:], lg[:, :DOUT], mx[:], None, AL.subtract)
            ex = p_small.tile([BL, DOUT], F32, name="ex", tag="lg")
            ssum = p_small.tile([BL, 1], F32, name="ssum_l", tag="lg")
            nc.scalar.activation(ex[:], tsh[:], AF.Exp, accum_out=ssum[:])
            lse = p_small.tile([BL, 1], F32, name="lse", tag="lg")
            nc.scalar.activation(lse[:], ssum[:], AF.Ln)
            res = p_small.tile([BL, DOUT], F32, name="res", tag="lg")
            nc.vector.tensor_scalar(res[:], tsh[:], lse[:], None, AL.subtract)
            nc.sync.dma_start(out_d.ap(), res[:])

    nc.compile()
    return nc


def prep_in_maps(inputs):
    """Host-side prep: slice per core, pad/retile weights. Returns in_maps."""
    tokens = np.asarray(inputs["tokens"]).astype(np.int32)
    lengths = np.asarray(inputs["lengths"]).astype(np.int32)
    f = lambda k: np.ascontiguousarray(np.asarray(inputs[k], dtype=np.float32))
    embed = f("embed")
    W0, b0 = f("W0"), f("b0")
    Wg, bg = f("Wg"), f("bg")
    Wgf, bgf = f("Wgf"), f("bgf")
    Wfi, bfi = f("Wfi"), f("bfi")
    Wgo, bgo = f("Wgo"), f("bgo")
    W1, b1 = f("W1"), f("b1")
    W2, b2 = f("W2"), f("b2")

    def tile_km(w, kc, mc):
        # [kc*128, mc*128] -> [mc, 128, kc, 128]: piece[m][p,k,c] = w[k*128+p, m*128+c]
        return np.ascontiguousarray(
            w.reshape(kc, 128, mc, 128).transpose(2, 1, 0, 3)
        )

    f8 = ml_dtypes.float8_e4m3

    def q8(w, scale):
        return np.clip(w * scale, -240.0, 240.0).astype(f8)

    wg_hhh = q8(tile_km(Wg[: 3 * H], KHH, GC), SW)
    wg_x_pad = np.zeros((EP4, 7 * H), np.float32)
    wg_x_pad[:E] = Wg[3 * H : 3 * H + E]
    wg_x = q8(tile_km(wg_x_pad, EC4, GC), SWX)
    # gg columns are pre-scaled by SH*SW so the selector matmul lands in the
    # fp8-scaled psum and the single evict descale applies uniformly
    gcat = np.concatenate([Wg[3 * H + E :], Wfi[:H]], axis=1) * (SH * SW)
    wg_gcat = np.ascontiguousarray(
        gcat.reshape(HC, 128, GGC, 512).transpose(0, 2, 1, 3)
    )
    wfi_hp = q8(
        np.ascontiguousarray(Wfi[H:].reshape(HC, 128, H).transpose(1, 0, 2)), SW
    )
    wgf = tile_km(Wgf, 2 * HC, HC)
    wgo = tile_km(Wgo, 2 * HC, HC)
    w0_pad = np.zeros((EP, H), np.float32)
    w0_pad[:E] = W0
    w0 = tile_km(w0_pad, EC, HC)
    w1 = tile_km(W1, HC, 2 * HC)
    w2p = np.zeros((2 * H, DP), np.float32)
    w2p[:, :DOUT] = W2
    b2p = np.zeros((DP,), np.float32)
    b2p[:DOUT] = b2
    w2 = np.ascontiguousarray(w2p.reshape(2 * HC, 128, DP).transpose(1, 0, 2))

    def t_bias(b):
        return np.ascontiguousarray(b.reshape(-1, 128).T)

    sel = np.zeros((128, N), np.float32)
    for e in range(BL):
        sel[e, e * L : (e + 1) * L] = 1.0
    ident = np.eye(128, dtype=np.float32)

    bf = ml_dtypes.bfloat16
    shared = dict(
        embed=embed, wg_hhh=wg_hhh, wg_x=wg_x,
        wg_gcat=wg_gcat.astype(bf), wfi_h=wfi_hp,
        wgf=wgf.astype(bf), wgo=wgo.astype(bf), w0=w0,
        w1=w1.astype(bf), w2=w2.astype(bf),
        bg_t=t_bias(bg), b0_t=t_bias(b0), bfi_t=t_bias(bfi), bgf_t=t_bias(bgf),
        bgo_t=t_bias(bgo), b1_t=t_bias(b1),
        b2_r=np.ascontiguousarray(np.tile(b2p[None, :], (BL, 1))),
        sel=sel, ident=ident,
    )

    in_maps = []
    for c in range(NCORES):
        sl = slice(c * BL, (c + 1) * BL)
        tok = tokens[sl]                                   # [BL, L]
        lens = np.maximum(lengths[sl].astype(np.float32), 1.0)
        mask = (np.arange(L)[None, :] < lengths[sl][:, None]).astype(np.float32)
        mask_rep = np.ascontiguousarray(
            np.broadcast_to(mask.reshape(1, N), (128, N))
        )
        invlen_rep = np.ascontiguousarray(
            np.broadcast_to((1.0 / lens).reshape(1, BL), (128, BL))
        )
        tok_idx = np.ascontiguousarray(tok.T.astype(np.int32))  # [L=128, BL]
        m = dict(shared)
        m.update(tok_idx=tok_idx, mask_rep=mask_rep, invlen_rep=invlen_rep)
        in_maps.append(m)
    return in_maps


_NC_CACHE = {}


def kernel(**inputs) -> np.ndarray:
    in_maps = prep_in_maps(inputs)
    if "nc" not in _NC_CACHE:
        _NC_CACHE["nc"] = build_nc()
    nc = _NC_CACHE["nc"]
    res = run_bass_kernel_spmd(nc, in_maps, core_ids=list(range(NCORES)))
    return np.concatenate([r["out"] for r in res.results], axis=0)


if __name__ == "__main__":
    nc = build_nc()
    print("built ok")

